# revision 6
# baseline (speedup 1.0000x reference)
"""nn_Attention_60266981097535 kernel — 8 NeuronCores via jax.pmap.

Sharding (per spec hint): 8 shards = (batch b in 0..3) x (query-row
half in 0..1), data-parallel across the 8 axon-tunneled trn2
NeuronCores.  Each shard computes the full per-batch q/k/R (needed
globally: k and the R Gram span all tokens) and its 512-row half of
the attention + output; no collectives needed.

Optimizations:
- adj is transferred as packed bits (np.packbits, 8 entries/byte):
  256 MiB of per-shard uint8 -> 32 MiB over the device link, unpacked
  on-device with shift/mask.
- diag(R) @ attn @ diag(R) commutes with leaky_relu (R = sigmoid > 0,
  leaky_relu positively homogeneous), so R is folded into q/k before
  the (T,T) score matmul — removes two O(H*T*T) elementwise passes.

Falls back to multithreaded CPU JAX if no accelerator is usable.
"""

import numpy as np

B, T, DIM, H = 4, 1024, 256, 8
D = DIM // H
HALF = T // 2
N_SHARDS = 8


def _shard_fn(shard_idx, x, adj_bits, Wq_g, Wk_g, Wv_g, Wq, Wk, Wv, Wkf,
              Wkf2, sparse_D, randomatrix):
    """x: (T, DIM) f32, adj_bits: (H, T, T//8) u8 (big-endian bit order).
    Returns output rows [s0:s0+HALF], s0 = (shard_idx % 2) * HALF."""
    import jax
    import jax.numpy as jnp

    scale = DIM ** (-0.5)
    shifts = jnp.arange(7, -1, -1, dtype=jnp.uint8)          # packbits 'big'
    adj_u8 = ((adj_bits[..., None] >> shifts) & jnp.uint8(1)).reshape(
        H, T, T)                                              # h t t 0/1
    adj_f = adj_u8.astype(jnp.float32)

    xh = x.reshape(T, H, D).transpose(1, 0, 2)                # h t d
    xq = jnp.einsum('htd,de->hte', xh, Wq_g)
    xk = jnp.einsum('htd,de->hte', xh, Wk_g)
    q_g = jax.nn.relu(jnp.einsum('hst,htd->hsd', adj_f, xq))
    k_g = jax.nn.relu(jnp.einsum('hst,htd->hsd', adj_f, xk))

    q = q_g.transpose(1, 0, 2).reshape(T, DIM) @ Wq           # t dim
    k = k_g.transpose(1, 0, 2).reshape(T, DIM) @ Wk

    R0 = jax.nn.gelu(jnp.concatenate([q, k], axis=-1) @ Wkf,
                     approximate=False)                        # t h
    R = jnp.einsum('th,tk->hk', R0, R0)                        # h h
    R = jnp.asarray(jax.nn.sigmoid((R @ Wkf2) / sparse_D))     # h t

    qh = q.reshape(T, H, D).transpose(1, 0, 2) * R[..., None]  # h t d
    kh = k.reshape(T, H, D).transpose(1, 0, 2) * R[..., None]

    s0 = (shard_idx % 2) * HALF
    qh_half = jax.lax.dynamic_slice_in_dim(qh, s0, HALF, axis=1)

    attn = jax.nn.leaky_relu(
        jnp.einsum('hld,htd->hlt', qh_half, kh) * scale)       # h half t
    attn = jnp.einsum('lh,hst->lst', randomatrix, attn)

    adj_half = jax.lax.dynamic_slice_in_dim(adj_u8, s0, HALF, axis=1)
    attn = jnp.where(adj_half > 0, attn, jnp.asarray(-1e12, jnp.float32))
    attn = jax.nn.softmax(attn, axis=-1)                       # h half t

    xv = jnp.einsum('htd,de->hte', xh, Wv_g)
    v = jax.nn.relu(jnp.einsum('hst,htd->hsd', attn, xv))      # h half d
    out = jax.nn.gelu(v.transpose(1, 0, 2).reshape(HALF, DIM) @ Wv,
                      approximate=False)
    return out


def _run_pmap(devs, x32, adj, weights):
    import jax
    from concurrent.futures import ThreadPoolExecutor

    n = N_SHARDS
    devs = devs[:n]
    f = jax.pmap(
        _shard_fn,
        in_axes=(0, 0, 0) + (0,) * 10,
        devices=devs,
    )
    # Per-device puts from threads overlap the tunnel's transfer latency;
    # per-batch packbits is pipelined against the transfers; the small
    # weights ship once per device in a single pytree put instead of
    # pmap's per-array broadcast.
    with ThreadPoolExecutor(n) as ex:
        wfuts = [ex.submit(jax.device_put,
                           tuple(weights) + (np.int32(i),), devs[i])
                 for i in range(n)]
        put_futs = [None] * n
        for b in range(B):
            bits_b = np.packbits(np.asarray(adj[b], np.uint8), axis=-1)
            for half in (0, 1):
                i = 2 * b + half
                put_futs[i] = ex.submit(
                    jax.device_put, (x32[b], bits_b), devs[i])
        pairs = [fu.result() for fu in put_futs]
        wrep = [fu.result() for fu in wfuts]
    xs = jax.device_put_sharded([p[0] for p in pairs], devs)
    adjs = jax.device_put_sharded([p[1] for p in pairs], devs)
    idxs = jax.device_put_sharded([wrep[i][-1] for i in range(n)], devs)
    ws = [jax.device_put_sharded([wrep[i][j] for i in range(n)], devs)
          for j in range(len(weights))]
    out_halves = np.asarray(f(idxs, xs, adjs, *ws))
    out = np.empty((B, T, DIM), np.float32)
    for i in range(n):
        b, hh = i // 2, i % 2
        out[b, hh * HALF:(hh + 1) * HALF] = out_halves[i]
    return out


def _run_cpu(x32, adj_bits, weights):
    import jax

    cpu = jax.devices('cpu')[0]
    fj = jax.jit(_shard_fn, static_argnums=(0,), backend='cpu')
    with jax.default_device(cpu):
        outs = []
        for bb in range(B):
            o0 = fj(0, x32[bb], adj_bits[bb], *weights)
            o1 = fj(1, x32[bb], adj_bits[bb], *weights)
            outs.append(np.concatenate([np.asarray(o0), np.asarray(o1)], 0))
    return np.stack(outs).astype(np.float32)


def kernel(x, adj, Wq_g, Wk_g, Wv_g, Wq, Wk, Wv, Wkf, Wkf2, sparse_D,
           randomatrix, label):
    import jax

    try:
        jax.config.update("jax_compilation_cache_dir",
                          "/tmp/jax_kcache_60266981097535")
        jax.config.update("jax_persistent_cache_min_entry_size_bytes", -1)
        jax.config.update("jax_persistent_cache_min_compile_time_secs", 0)
    except Exception:
        pass

    weights = tuple(
        np.asarray(w, np.float32)
        for w in (Wq_g, Wk_g, Wv_g, Wq, Wk, Wv, Wkf, Wkf2, sparse_D,
                  randomatrix))
    x32 = np.asarray(x, np.float32)
    adj = np.asarray(adj)

    try:
        devs = [d for d in jax.devices() if d.platform != 'cpu']
        if len(devs) >= N_SHARDS:
            return _run_pmap(devs, x32, adj, weights)
    except Exception:
        pass
    adj_bits = np.packbits(np.asarray(adj, np.uint8), axis=-1)  # B H T T/8
    return _run_cpu(x32, adj_bits, weights)
